# revision 10
# baseline (speedup 1.0000x reference)
"""CRF negative-log-likelihood kernel for Trainium2 (8 NeuronCores, SPMD).

Strategy
--------
Data-parallel over batch: core k owns sequences [64k, 64k+64).

The CRF forward (log-partition) recurrence is run in the exp domain:
    w_{s}  = (E^T w_{s-1}) * Fhat_s          (per sequence, T=64-dim state)
with E = exp(transitions) and Fhat_s = exp(feats_s - c), c = log(64)+0.5 a
global constant that keeps the state magnitude O(1) (the exact per-step
offsets are reconstructed on the host as (L-1)*c).

To halve the serial depth, each sequence is split at M in {127, 255}: the
forward recurrence runs from s=0 up to s=M while the backward (beta)
recurrence runs from s=L-1 down to s=M+1 — both simultaneously, stacked on
the 128 SBUF partitions (fwd tags on partitions 0:64, bwd on 64:128) with a
block-diagonal stationary weight blockdiag(E, E^T).  256 macro-steps total,
each = one 128x128->[128,64] bf16 matmul (PE) + one elementwise multiply
(DVE) with a precomputed schedule tensor Fsched.

Variable lengths are handled entirely in host-side data prep: feats are
pre-permuted into schedule order (dead slots zero), and chain "boots" are
injected as extra accumulating matmuls at fixed steps reading host-built
init tensors — so the device program is input-independent and identical
across cores (compiled once).

The final per-sequence sums S_b = sum_t wfwd_M[t] * (E wbwd_cap)[t] are
computed on device and shipped out ([1,64] per core); the host finishes with
Z_b = log(S_b) + (L_b-1)*c, sums them, and subtracts the gold score (a cheap
O(B*S) gather done in numpy float64).
"""
import sys

for _p in ("/opt/trn_rl_repo",):
    if _p not in sys.path:
        sys.path.insert(0, _p)

import numpy as np
import ml_dtypes

BF16 = ml_dtypes.bfloat16

B, S, T = 512, 512, 64
N_CORES = 8
SEQ_PER_CORE = B // N_CORES          # 64
NSTEP = 256
START, STOP = T - 2, T - 1
C_NORM = float(np.log(64.0) + 0.5)
NBOOT_BWD = 129                      # bwd boot window: steps 1..129

_PROG_CACHE = {}


def _build_program():
    import concourse.bacc as bacc
    import concourse.mybir as mybir
    from concourse.tile import TileContext

    f32 = mybir.dt.float32
    bf16 = mybir.dt.bfloat16
    n = SEQ_PER_CORE

    nc = bacc.Bacc()
    feats_sched = nc.declare_dram_parameter(
        "feats_sched", [n * NSTEP, 128], f32, isOutput=False)
    w_in = nc.declare_dram_parameter("w_blocks", [128, 128], bf16, isOutput=False)
    inj_a = nc.declare_dram_parameter("inj_a", [T, n], bf16, isOutput=False)
    inj_b = nc.declare_dram_parameter("inj_b", [T, n], bf16, isOutput=False)
    inj_bwd = nc.declare_dram_parameter(
        "inj_bwd", [T, NBOOT_BWD * n], bf16, isOutput=False)
    out_s = nc.declare_dram_parameter("out_s", [1, n], f32, isOutput=True)

    EXP = mybir.ActivationFunctionType.Exp

    with TileContext(nc) as tc:
        with (
            tc.tile_pool(name="persist", bufs=1) as pp,
            tc.tile_pool(name="stage", bufs=3) as sp,
            tc.tile_pool(name="dram", bufs=1, space="DRAM") as dp,
            tc.tile_pool(name="psum", bufs=2, space="PSUM") as psp,
        ):
            # [tag-dims, window, slot, col-in-window]: each window's
            # transpose output is contiguous (the DMA xbar ignores
            # strided 3D out APs and writes contiguously)
            Fs = pp.tile([128, NSTEP // 16, n, 16], bf16)
            Z = pp.tile([128, n], bf16)             # scan state (fwd | bwd)
            W = pp.tile([128, 128], bf16)           # blockdiag(E, E^T)
            IA = pp.tile([T, n], bf16)
            IB = pp.tile([T, n], bf16)
            IBW = pp.tile([128, NBOOT_BWD * n], bf16)
            ONES = pp.tile([T, 1], f32)
            PROD = pp.tile([T, n], f32)
            OUT = pp.tile([1, n], f32)

            nc.sync.dma_start(W[:], w_in[:])
            nc.sync.dma_start(IA[:], inj_a[:])
            nc.sync.dma_start(IB[:], inj_b[:])
            nc.sync.dma_start(IBW[64:128, :], inj_bwd[:])
            nc.vector.memset(Z[:], 0.0)
            nc.vector.memset(ONES[:], 1.0)

            # ---- precompute Fsched: exp(feats_sched - c) transposed ----
            # feats_sched rows are window-major: row = w*1024 + v*16 + c_i
            # (slot v, step-col 16w + c_i), cols = 128 tag-dims
            # (fwd seq tags 0:64 | bwd seq tags 64:128).  Each 16-step
            # window: contiguous load -> exp -> bf16 scratch -> one big
            # DMA-xbar transpose into Fsched (so consumers wait on exactly
            # one DMA each).
            scratch = dp.tile([n * NSTEP, 128], bf16)
            fsv = feats_sched[:].rearrange("(w p g) t -> w p (g t)", p=128, g=8)
            scv = scratch[:].rearrange("(w p g) t -> w p (g t)", p=128, g=8)
            for w in range(NSTEP // 16):
                stg = sp.tile([128, 1024], f32, tag="stg_in")
                nc.sync.dma_start(stg[:], fsv[w])
                # dedicated mid tile per window: the exp never carries a
                # write-after-read wait (ISA sync-slot budget on ACT is tiny)
                mid = pp.tile([128, 1024], bf16, tag=f"mid{w}")
                nc.scalar.activation(mid[:], stg[:], EXP)
                nc.sync.dma_start(scv[w], mid[:])
                nc.sync.dma_start_transpose(
                    Fs[:, w], scratch[w * 1024:(w + 1) * 1024, :])

            # ---- the 256-step meet-in-the-middle scan ----
            sink = pp.tile([1, 16], bf16)
            for i in range(1, NSTEP + 1):
                if (i - 1) % 16 == 0:
                    # absorb the Fsched-transpose DMA wait on a cheap DVE op
                    nc.vector.tensor_copy(
                        sink[:], Fs[0:1, (i - 1) // 16, 0:1, :])
                ps = psp.tile([128, n], mybir.dt.float32, tag="scanps")
                has_fa = i == 2
                has_fb = i == 130
                has_bw = i <= NBOOT_BWD
                n_mm = 1 + has_fa + has_fb + has_bw
                k = 1
                nc.tensor.matmul(ps[:], W[:], Z[:], start=True, stop=(k == n_mm))
                if has_fa:
                    k += 1
                    nc.tensor.matmul(ps[0:64, :], W[0:64, 0:64], IA[:],
                                     start=False, stop=(k == n_mm))
                if has_fb:
                    k += 1
                    nc.tensor.matmul(ps[0:64, :], W[0:64, 0:64], IB[:],
                                     start=False, stop=(k == n_mm))
                if has_bw:
                    k += 1
                    nc.tensor.matmul(ps[64:128, :], W[64:128, 64:128],
                                     IBW[64:128, (i - 1) * n:i * n],
                                     start=False, stop=(k == n_mm))
                nc.vector.tensor_mul(
                    Z[:], ps[:], Fs[:, (i - 1) // 16, :, (i - 1) % 16])

            # ---- final combine: S = sum_t Zfwd * (E @ Zbwd) ----
            psD = psp.tile([T, n], mybir.dt.float32, tag="psD")
            nc.tensor.matmul(psD[:], W[64:128, 64:128], Z[64:128, :],
                             start=True, stop=True)
            nc.vector.tensor_mul(PROD[:], psD[:], Z[0:64, :])
            psS = psp.tile([1, n], mybir.dt.float32, tag="psS")
            nc.tensor.matmul(psS[:], ONES[:], PROD[:], start=True, stop=True)
            nc.vector.tensor_copy(OUT[:], psS[:])
            nc.sync.dma_start(out_s[:], OUT[:])

    nc.finalize()
    return nc


def _get_program():
    if "nc" not in _PROG_CACHE:
        _PROG_CACHE["nc"] = _build_program()
    return _PROG_CACHE["nc"]


def _host_prep(feats, lengths, transitions):
    """Build per-core input maps. feats [B,S,T] f32, lengths [B] int."""
    trans64 = transitions.astype(np.float64)
    E = np.exp(trans64).astype(np.float32)
    Wb = np.zeros((128, 128), np.float32)
    Wb[0:64, 0:64] = E
    Wb[64:128, 64:128] = E.T
    Wb = Wb.astype(BF16)

    n = SEQ_PER_CORE
    in_maps = []
    for core in range(N_CORES):
        sl = slice(core * n, (core + 1) * n)
        fc = feats[sl]                       # [n, S, T]
        lc = lengths[sl]
        fs = np.full((n, NSTEP, 128), -C_NORM, np.float32)
        ia = np.zeros((T, n), np.float32)
        ib = np.zeros((T, n), np.float32)
        ibw = np.zeros((T, NBOOT_BWD * n), np.float32)
        for v in range(n):
            L = int(lc[v])
            M = 127 if L <= 383 else 255
            s_arr = np.arange(1, M + 1)
            fs[v, s_arr + 255 - M, 0:64] = fc[v, s_arr, :] - C_NORM
            s_arr = np.arange(M + 1, L)
            fs[v, 256 + M - s_arr, 64:128] = fc[v, s_arr, :] - C_NORM
            w0 = np.exp(fc[v, 0, :].astype(np.float64) + trans64[START, :])
            (ia if M == 255 else ib)[:, v] = w0.astype(np.float32)
            i0b = 258 + M - L
            ibw[STOP, (i0b - 1) * n + v] = 1.0
        # window-major layout: row = w*1024 + v*16 + c_i  (c = 16w + c_i)
        fs_sched = (fs.reshape(n, NSTEP // 16, 16, 128)
                      .transpose(1, 0, 2, 3)
                      .reshape(n * NSTEP, 128))
        in_maps.append({
            "feats_sched": fs_sched,
            "w_blocks": Wb,
            "inj_a": ia.astype(BF16),
            "inj_b": ib.astype(BF16),
            "inj_bwd": ibw.astype(BF16),
        })
    return in_maps


def _gold_score(feats, mask, tags, transitions):
    t64 = transitions.astype(np.float64)
    prev = np.concatenate(
        [np.full((B, 1), START, dtype=tags.dtype), tags[:, :-1]], axis=1)
    emit = np.take_along_axis(
        feats, tags[:, :, None].astype(np.int64), axis=2)[:, :, 0]
    tg = emit.astype(np.float64) + t64[prev, tags]
    gold = np.where(mask, tg, 0.0).sum()
    lengths = mask.sum(axis=1).astype(np.int64)
    end_ids = np.take_along_axis(tags, (lengths - 1)[:, None].astype(tags.dtype),
                                 axis=1)[:, 0]
    return gold + t64[end_ids, STOP].sum()


def kernel(feats, mask, tags, transitions, _trace=False):
    from concourse.bass_utils import run_bass_kernel_spmd

    feats = np.asarray(feats, dtype=np.float32)
    mask = np.asarray(mask)
    tags = np.asarray(tags)
    transitions = np.asarray(transitions, dtype=np.float32)
    lengths = mask.astype(np.int64).sum(axis=1)

    nc = _get_program()
    in_maps = _host_prep(feats, lengths, transitions)
    res = run_bass_kernel_spmd(nc, in_maps, core_ids=list(range(N_CORES)),
                               trace=_trace)
    _PROG_CACHE["last_result"] = res

    svec = np.concatenate(
        [res.results[c]["out_s"][0].astype(np.float64) for c in range(N_CORES)])
    zb = np.log(svec) + (lengths.astype(np.float64) - 1.0) * C_NORM
    forward_score = zb.sum()
    gold = _gold_score(feats, mask, tags, transitions)
    return np.float32(forward_score - gold)
